# revision 7
# baseline (speedup 1.0000x reference)
"""L2 (spectral) contrastive loss on 8 Trainium2 NeuronCores.

Math: with G_x = x.T @ x and G_y = y.T @ y (both [D, D]),
    sum_{i,j} <x_i, y_j>^2 = tr(G_x @ G_y) = sum(G_x * G_y)
so the loss needs only the two Gram matrices (2*N*D^2 MACs) instead of the
[N, N] pairwise product (N^2*D MACs).

v2 pipeline (vs the v1 single fp16 AllReduce of both packed Grams):
  - rows of x and y are split across the 8 cores; each core computes
    partial Grams over its 1024 rows (bf16 matmuls, fp32 PSUM, upper
    triangle only, with the x-side off-diagonal slabs pre-scaled by 2 so
    a plain elementwise dot of the packed buffers gives the full-triangle
    weighted sum).
  - TWO ReduceScatters (one per Gram, 688 KB fp16 each) instead of one
    1.38 MB AllReduce: RS#1 (G_x) is triggered as soon as G_x is packed
    and overlaps the G_y compute, absorbing cross-core launch skew.
    The pack is permuted on its way to DRAM so each rank's RS slice is a
    dense [128, 336] tile (full-width DVE dot, single DMA back).
  - each rank dots its slice locally; the per-core scalars (slice dot,
    sum z_i, sum z_i^2, where z_i = <x_i, y_i>) ride a tiny fp32
    AllGather; every core redundantly computes
      loss = (dot - sum z^2)/(N*(N-1)) - (2/N)*sum z
    and core 0's output is returned.
  - fp32->bf16 casts of x and the G_y pack copies run on the (otherwise
    idle) scalar/Act engine so the DVE stays off the critical path.
"""
import numpy as np
from contextlib import ExitStack

from concourse import bacc, tile, mybir
from concourse.bass_utils import run_bass_kernel_spmd

N_CORES = 8
N, D = 8192, 768
ROWS = N // N_CORES          # 1024 rows per core
P = 128                      # SBUF partitions
KCH = ROWS // P              # 8 contraction chunks per core
MS = D // P                  # 6 output slabs per Gram

# upper-triangle slab widths and packed column offsets
WIDTHS = [D - P * m for m in range(MS)]              # [768,640,512,384,256,128]
COFF = [sum(WIDTHS[:m]) for m in range(MS)]          # prefix offsets
GCOLS = sum(WIDTHS)                                  # 2688 per Gram
SLICE = GCOLS // N_CORES                             # 336 cols per RS slice

F32 = mybir.dt.float32
F16 = mybir.dt.float16
BF16 = mybir.dt.bfloat16

RG = [list(range(N_CORES))]

_CACHE = {}


def _free_chunks(width):
    """Split [0, width) at the 512-column PSUM bank boundary."""
    if width <= 512:
        return [(0, width)]
    return [(0, 512), (512, width)]


def _build():
    nc = bacc.Bacc("TRN2", target_bir_lowering=False, debug=False,
                   num_devices=N_CORES)
    x_ap = nc.dram_tensor("x", [ROWS, D], F32, kind="ExternalInput").ap()
    y_ap = nc.dram_tensor("y", [ROWS, D], F32, kind="ExternalInput").ap()
    loss_ap = nc.dram_tensor("loss", [1, 1], F32, kind="ExternalOutput").ap()

    inv_nn1 = 1.0 / (float(N) * (N - 1))

    with tile.TileContext(nc) as tc:
        with ExitStack() as ctx:
            sb = ctx.enter_context(tc.tile_pool(name="sb", bufs=1))
            ps = ctx.enter_context(tc.tile_pool(name="ps", bufs=1, space="PSUM"))
            dram = ctx.enter_context(tc.tile_pool(name="dram", bufs=1, space="DRAM"))

            # ---- load inputs: [1024, 768] -> [128p, 8k, 768], x first ----
            xt = sb.tile([P, KCH, D], F32)
            yt = sb.tile([P, KCH, D], F32)
            xr = x_ap.rearrange("(n p) d -> p n d", p=P)
            yr = y_ap.rearrange("(n p) d -> p n d", p=P)
            for k in range(KCH):
                nc.sync.dma_start(xt[:, k, :], xr[:, k, :])
            for k in range(KCH):
                nc.sync.dma_start(yt[:, k, :], yr[:, k, :])

            # ---- casts: x on DVE, y on Act (scalar) ----
            xb = sb.tile([P, KCH, D], BF16)
            yb = sb.tile([P, KCH, D], BF16)
            for k in range(KCH):
                nc.vector.tensor_copy(xb[:, k, :], xt[:, k, :])
            for k in range(KCH):
                nc.scalar.copy(yb[:, k, :], yt[:, k, :])

            ones = sb.tile([P, 1], F32)
            nc.vector.memset(ones[:], 1.0)

            # ---- Grams: upper-triangle slabs, bf16 matmul, fp32 PSUM.
            # Pack copies apply the triangle weighting: the first 128
            # (block-diagonal) columns of each slab are copied with scale 1,
            # the strict-upper remainder with scale 2 (x side only).
            pack_x = sb.tile([P, GCOLS], F16)
            pack_y = sb.tile([P, GCOLS], F16)

            def gram(src, pack, scaled):
                for m in range(MS):
                    w = WIDTHS[m]
                    slab = ps.tile([P, w], F32, tag="slab", bufs=3,
                                   padded_shape=[P, 768], name=f"slab{m}")
                    for (c0, c1) in _free_chunks(w):
                        for k in range(KCH):
                            nc.tensor.matmul(
                                slab[:, c0:c1],
                                src[:, k, P * m:P * (m + 1)],
                                src[:, k, P * m + c0:P * m + c1],
                                start=(k == 0),
                                stop=(k == KCH - 1),
                            )
                    off = COFF[m]
                    if scaled:  # x side: DVE, with x2 on strict-upper cols
                        nc.vector.tensor_copy(pack[:, off:off + P],
                                              slab[:, 0:P])
                        if w > P:
                            nc.vector.tensor_scalar_mul(
                                pack[:, off + P:off + w], slab[:, P:w], 2.0)
                    else:       # y side: Act engine, plain copy
                        nc.scalar.copy(pack[:, off:off + P], slab[:, 0:P])
                        if w > P:
                            nc.scalar.copy(pack[:, off + P:off + w],
                                           slab[:, P:w])

            # ---- G_x -> permuted DRAM -> ReduceScatter #1 ----
            gram(xb, pack_x, scaled=True)
            cin_x = dram.tile([N_CORES, P, SLICE], F16)
            rs_x = dram.tile([P, SLICE], F16)
            nc.sync.dma_start(cin_x.rearrange("j p c -> p j c"),
                              pack_x.rearrange("p (j c) -> p j c", j=N_CORES))
            nc.gpsimd.collective_compute(
                "ReduceScatter", mybir.AluOpType.add, replica_groups=RG,
                ins=[cin_x.opt()], outs=[rs_x.opt()],
            )

            # ---- G_y -> permuted DRAM -> ReduceScatter #2 ----
            gram(yb, pack_y, scaled=False)
            cin_y = dram.tile([N_CORES, P, SLICE], F16)
            rs_y = dram.tile([P, SLICE], F16)
            nc.sync.dma_start(cin_y.rearrange("j p c -> p j c"),
                              pack_y.rearrange("p (j c) -> p j c", j=N_CORES))
            nc.gpsimd.collective_compute(
                "ReduceScatter", mybir.AluOpType.add, replica_groups=RG,
                ins=[cin_y.opt()], outs=[rs_y.opt()],
            )

            # ---- diagonal terms z_i = <x_i, y_i> (DVE, off critical path)
            # zd columns: [0]=sum z, [1]=sum z^2, [2]=slice dot
            zd = sb.tile([P, 3], F32)
            zcols = sb.tile([P, KCH], F32)
            zscr = sb.tile([P, D], F32)
            for k in range(KCH):
                nc.vector.scalar_tensor_tensor(
                    zscr[:], xb[:, k, :], 1.0, yb[:, k, :],
                    mybir.AluOpType.mult, mybir.AluOpType.mult,
                    accum_out=zcols[:, k:k + 1],
                )
            zsq = sb.tile([P, KCH], F32)
            nc.vector.tensor_mul(zsq[:], zcols[:], zcols[:])
            nc.vector.tensor_reduce(zd[:, 0:1], zcols[:], mybir.AxisListType.X,
                                    mybir.AluOpType.add)
            nc.vector.tensor_reduce(zd[:, 1:2], zsq[:], mybir.AxisListType.X,
                                    mybir.AluOpType.add)

            # ---- local dot of this rank's RS slices ----
            ax = sb.tile([P, SLICE], F16)
            by = sb.tile([P, SLICE], F16)
            nc.sync.dma_start(ax[:], rs_x[:])
            nc.sync.dma_start(by[:], rs_y[:])
            dscr = sb.tile([P, SLICE], F32)
            nc.vector.scalar_tensor_tensor(
                dscr[:], ax[:], 1.0, by[:],
                mybir.AluOpType.mult, mybir.AluOpType.mult,
                accum_out=zd[:, 2:3],
            )

            # ---- partition-reduce the three columns via PE (ones^T @ zd)
            pz = ps.tile([1, 3], F32, tag="pz", bufs=1)
            nc.tensor.matmul(pz[0:1, 0:3], ones[:, 0:1], zd[:, 0:3],
                             start=True, stop=True)
            scg = sb.tile([1, 128], F32)
            nc.vector.memset(scg[:], 0.0)
            nc.vector.tensor_copy(scg[0:1, 0:3], pz[0:1, 0:3])

            # ---- tiny fp32 AllGather of per-core scalars ----
            cin_g = dram.tile([1, 128], F32)
            gout = dram.tile([N_CORES, 128], F32)
            nc.sync.dma_start(cin_g[:], scg[:])
            nc.gpsimd.collective_compute(
                "AllGather", mybir.AluOpType.bypass, replica_groups=RG,
                ins=[cin_g.opt()], outs=[gout.opt()],
            )
            gg = sb.tile([N_CORES, 128], F32)
            nc.sync.dma_start(gg[:], gout[:])

            # ---- finale: loss = inv_nn1*(dot - sum z^2) - (2/N)*sum z ----
            tot = ps.tile([1, 3], F32, tag="tot", bufs=1)
            nc.tensor.matmul(tot[0:1, 0:3], ones[0:N_CORES, 0:1],
                             gg[0:N_CORES, 0:3], start=True, stop=True)
            tots = sb.tile([1, 3], F32)
            nc.vector.tensor_copy(tots[:], tot[0:1, 0:3])
            d1 = sb.tile([1, 1], F32)
            nc.vector.tensor_sub(d1[:], tots[0:1, 2:3], tots[0:1, 1:2])
            d2 = sb.tile([1, 1], F32)
            nc.vector.tensor_scalar_mul(d2[:], d1[:], inv_nn1)
            res = sb.tile([1, 1], F32)
            nc.vector.scalar_tensor_tensor(
                res[:], tots[0:1, 0:1], -2.0 / N, d2[:],
                mybir.AluOpType.mult, mybir.AluOpType.add,
            )
            nc.sync.dma_start(loss_ap[:], res[:])

    nc.compile()
    return nc


def _get_nc():
    if "nc" not in _CACHE:
        _CACHE["nc"] = _build()
    return _CACHE["nc"]


def _run(x, y, trace=False, **trace_kwargs):
    nc = _get_nc()
    x = np.ascontiguousarray(np.asarray(x, dtype=np.float32))
    y = np.ascontiguousarray(np.asarray(y, dtype=np.float32))
    assert x.shape == (N, D) and y.shape == (N, D)
    in_maps = [
        {"x": x[c * ROWS:(c + 1) * ROWS], "y": y[c * ROWS:(c + 1) * ROWS]}
        for c in range(N_CORES)
    ]
    res = run_bass_kernel_spmd(nc, in_maps, list(range(N_CORES)), trace=trace,
                               **trace_kwargs)
    loss = np.float32(res.results[0]["loss"][0, 0])
    return np.asarray(loss, dtype=np.float32).reshape(()), res


def kernel(x, y):
    out, _ = _run(x, y, trace=False)
    return out


# revision 9
# speedup vs baseline: 1.0676x; 1.0676x over previous
"""L2 (spectral) contrastive loss on 8 Trainium2 NeuronCores.

Math: with G_x = x.T @ x and G_y = y.T @ y (both [D, D]),
    sum_{i,j} <x_i, y_j>^2 = tr(G_x @ G_y) = sum(G_x * G_y)
so the loss needs only the two Gram matrices (2*N*D^2 MACs) instead of the
[N, N] pairwise product (N^2*D MACs).

v2 pipeline (vs the v1 single fp16 AllReduce of both packed Grams):
  - rows of x and y are split across the 8 cores; each core computes
    partial Grams over its 1024 rows (bf16 matmuls, fp32 PSUM, upper
    triangle only, with the x-side off-diagonal slabs pre-scaled by 2 so
    a plain elementwise dot of the packed buffers gives the full-triangle
    weighted sum).
  - TWO ReduceScatters (one per Gram, 688 KB fp16 each) instead of one
    1.38 MB AllReduce: RS#1 (G_x) is triggered as soon as G_x is packed
    and overlaps the G_y compute, absorbing cross-core launch skew.
    The pack is permuted on its way to DRAM so each rank's RS slice is a
    dense [128, 336] tile (full-width DVE dot, single DMA back).
  - each rank dots its slice locally; the per-core scalars (slice dot,
    sum z_i, sum z_i^2, where z_i = <x_i, y_i>) ride a tiny fp32
    AllGather; every core redundantly computes
      loss = (dot - sum z^2)/(N*(N-1)) - (2/N)*sum z
    and core 0's output is returned.
  - fp32->bf16 casts of x and the G_y pack copies run on the (otherwise
    idle) scalar/Act engine so the DVE stays off the critical path.
"""
import numpy as np
from contextlib import ExitStack

from concourse import bacc, tile, mybir
from concourse.bass_utils import run_bass_kernel_spmd

N_CORES = 8
N, D = 8192, 768
ROWS = N // N_CORES          # 1024 rows per core
P = 128                      # SBUF partitions
KCH = ROWS // P              # 8 contraction chunks per core
MS = D // P                  # 6 output slabs per Gram

# upper-triangle slab widths and packed column offsets
WIDTHS = [D - P * m for m in range(MS)]              # [768,640,512,384,256,128]
COFF = [sum(WIDTHS[:m]) for m in range(MS)]          # prefix offsets
GCOLS = sum(WIDTHS)                                  # 2688 per Gram
SLICE = GCOLS // N_CORES                             # 336 cols per RS slice

F32 = mybir.dt.float32
F16 = mybir.dt.float16
BF16 = mybir.dt.bfloat16

RG = [list(range(N_CORES))]

_CACHE = {}


def _free_chunks(width):
    """Split [0, width) at the 512-column PSUM bank boundary."""
    if width <= 512:
        return [(0, width)]
    return [(0, 512), (512, width)]


def _build():
    nc = bacc.Bacc("TRN2", target_bir_lowering=False, debug=False,
                   num_devices=N_CORES)
    x_ap = nc.dram_tensor("x", [ROWS, D], F32, kind="ExternalInput").ap()
    y_ap = nc.dram_tensor("y", [ROWS, D], F32, kind="ExternalInput").ap()
    loss_ap = nc.dram_tensor("loss", [1, 1], F32, kind="ExternalOutput").ap()

    inv_nn1 = 1.0 / (float(N) * (N - 1))

    with tile.TileContext(nc) as tc:
        with ExitStack() as ctx:
            sb = ctx.enter_context(tc.tile_pool(name="sb", bufs=1))
            ps = ctx.enter_context(tc.tile_pool(name="ps", bufs=1, space="PSUM"))
            dram = ctx.enter_context(tc.tile_pool(name="dram", bufs=1, space="DRAM"))

            # ---- load inputs: [1024, 768] -> [128p, 8k, 768], x first ----
            xt = sb.tile([P, KCH, D], F32)
            yt = sb.tile([P, KCH, D], F32)
            xr = x_ap.rearrange("(n p) d -> p n d", p=P)
            yr = y_ap.rearrange("(n p) d -> p n d", p=P)
            for k in range(KCH):
                nc.sync.dma_start(xt[:, k, :], xr[:, k, :])
            for k in range(KCH):
                nc.sync.dma_start(yt[:, k, :], yr[:, k, :])

            # ---- casts: x on DVE, y on Act (scalar) ----
            xb = sb.tile([P, KCH, D], BF16)
            yb = sb.tile([P, KCH, D], BF16)
            for k in range(KCH):
                nc.vector.tensor_copy(xb[:, k, :], xt[:, k, :])
            for k in range(KCH):
                nc.scalar.copy(yb[:, k, :], yt[:, k, :])

            ones = sb.tile([P, 1], F32)
            nc.vector.memset(ones[:], 1.0)

            # ---- Grams: upper-triangle slabs, bf16 matmul, fp32 PSUM.
            # Pack copies apply the triangle weighting: the first 128
            # (block-diagonal) columns of each slab are copied with scale 1,
            # the strict-upper remainder with scale 2 (x side only).
            pack_x = sb.tile([P, GCOLS], F16)
            pack_y = sb.tile([P, GCOLS], F16)

            def gram(src, pack, scaled):
                for m in range(MS):
                    w = WIDTHS[m]
                    slab = ps.tile([P, w], F32, tag="slab", bufs=3,
                                   padded_shape=[P, 768], name=f"slab{m}")
                    for (c0, c1) in _free_chunks(w):
                        for k in range(KCH):
                            nc.tensor.matmul(
                                slab[:, c0:c1],
                                src[:, k, P * m:P * (m + 1)],
                                src[:, k, P * m + c0:P * m + c1],
                                start=(k == 0),
                                stop=(k == KCH - 1),
                            )
                    off = COFF[m]
                    if scaled:  # x side: DVE, with x2 on strict-upper cols
                        nc.vector.tensor_copy(pack[:, off:off + P],
                                              slab[:, 0:P])
                        if w > P:
                            nc.vector.tensor_scalar_mul(
                                pack[:, off + P:off + w], slab[:, P:w], 2.0)
                    else:       # y side: Act engine, plain copy
                        nc.scalar.copy(pack[:, off:off + P], slab[:, 0:P])
                        if w > P:
                            nc.scalar.copy(pack[:, off + P:off + w],
                                           slab[:, P:w])

            # ---- both Grams -> one permuted DRAM buffer -> single
            # ReduceScatter. Rank r's flat slice is cin[r] = [2P, SLICE]:
            # rows 0:P hold its G_x column-block, rows P:2P its G_y block,
            # so the local dot pairs matching columns. ----
            gram(xb, pack_x, scaled=True)
            gram(yb, pack_y, scaled=False)
            cin = dram.tile([N_CORES, 2 * P, SLICE], F16)
            rso = dram.tile([2 * P, SLICE], F16)
            nc.sync.dma_start(cin[:, 0:P, :].rearrange("j p c -> p j c"),
                              pack_x.rearrange("p (j c) -> p j c", j=N_CORES))
            nc.sync.dma_start(cin[:, P:2 * P, :].rearrange("j p c -> p j c"),
                              pack_y.rearrange("p (j c) -> p j c", j=N_CORES))
            nc.gpsimd.collective_compute(
                "ReduceScatter", mybir.AluOpType.add, replica_groups=RG,
                ins=[cin.opt()], outs=[rso.opt()],
            )

            # ---- diagonal terms z_i = <x_i, y_i> (DVE, off critical path)
            # zd columns: [0]=sum z, [1]=sum z^2, [2]=slice dot
            zd = sb.tile([P, 3], F32)
            zcols = sb.tile([P, KCH], F32)
            zscr = sb.tile([P, D], F32)
            for k in range(KCH):
                nc.vector.scalar_tensor_tensor(
                    zscr[:], xb[:, k, :], 1.0, yb[:, k, :],
                    mybir.AluOpType.mult, mybir.AluOpType.mult,
                    accum_out=zcols[:, k:k + 1],
                )
            zsq = sb.tile([P, KCH], F32)
            nc.vector.tensor_mul(zsq[:], zcols[:], zcols[:])
            nc.vector.tensor_reduce(zd[:, 0:1], zcols[:], mybir.AxisListType.X,
                                    mybir.AluOpType.add)
            nc.vector.tensor_reduce(zd[:, 1:2], zsq[:], mybir.AxisListType.X,
                                    mybir.AluOpType.add)

            # ---- local dot of this rank's RS slices ----
            ax = sb.tile([P, SLICE], F16)
            by = sb.tile([P, SLICE], F16)
            nc.sync.dma_start(ax[:], rso[0:P, :])
            nc.scalar.dma_start(by[:], rso[P:2 * P, :])
            dscr = sb.tile([P, SLICE], F32)
            nc.vector.scalar_tensor_tensor(
                dscr[:], ax[:], 1.0, by[:],
                mybir.AluOpType.mult, mybir.AluOpType.mult,
                accum_out=zd[:, 2:3],
            )

            # ---- partition-reduce the three columns via PE (ones^T @ zd)
            pz = ps.tile([1, 3], F32, tag="pz", bufs=1)
            nc.tensor.matmul(pz[0:1, 0:3], ones[:, 0:1], zd[:, 0:3],
                             start=True, stop=True)
            scg = sb.tile([1, 128], F32)
            nc.vector.memset(scg[:], 0.0)
            nc.vector.tensor_copy(scg[0:1, 0:3], pz[0:1, 0:3])

            # ---- tiny fp32 AllGather of per-core scalars ----
            cin_g = dram.tile([1, 128], F32)
            gout = dram.tile([N_CORES, 128], F32)
            nc.sync.dma_start(cin_g[:], scg[:])
            nc.gpsimd.collective_compute(
                "AllGather", mybir.AluOpType.bypass, replica_groups=RG,
                ins=[cin_g.opt()], outs=[gout.opt()],
            )
            gg = sb.tile([N_CORES, 128], F32)
            nc.sync.dma_start(gg[:], gout[:])

            # ---- finale: loss = inv_nn1*(dot - sum z^2) - (2/N)*sum z ----
            tot = ps.tile([1, 3], F32, tag="tot", bufs=1)
            nc.tensor.matmul(tot[0:1, 0:3], ones[0:N_CORES, 0:1],
                             gg[0:N_CORES, 0:3], start=True, stop=True)
            tots = sb.tile([1, 3], F32)
            nc.vector.tensor_copy(tots[:], tot[0:1, 0:3])
            d1 = sb.tile([1, 1], F32)
            nc.vector.tensor_sub(d1[:], tots[0:1, 2:3], tots[0:1, 1:2])
            d2 = sb.tile([1, 1], F32)
            nc.vector.tensor_scalar_mul(d2[:], d1[:], inv_nn1)
            res = sb.tile([1, 1], F32)
            nc.vector.scalar_tensor_tensor(
                res[:], tots[0:1, 0:1], -2.0 / N, d2[:],
                mybir.AluOpType.mult, mybir.AluOpType.add,
            )
            nc.sync.dma_start(loss_ap[:], res[:])

    nc.compile()
    return nc


def _get_nc():
    if "nc" not in _CACHE:
        _CACHE["nc"] = _build()
    return _CACHE["nc"]


def _run(x, y, trace=False, **trace_kwargs):
    nc = _get_nc()
    x = np.ascontiguousarray(np.asarray(x, dtype=np.float32))
    y = np.ascontiguousarray(np.asarray(y, dtype=np.float32))
    assert x.shape == (N, D) and y.shape == (N, D)
    in_maps = [
        {"x": x[c * ROWS:(c + 1) * ROWS], "y": y[c * ROWS:(c + 1) * ROWS]}
        for c in range(N_CORES)
    ]
    res = run_bass_kernel_spmd(nc, in_maps, list(range(N_CORES)), trace=trace,
                               **trace_kwargs)
    loss = np.float32(res.results[0]["loss"][0, 0])
    return np.asarray(loss, dtype=np.float32).reshape(()), res


def kernel(x, y):
    out, _ = _run(x, y, trace=False)
    return out
